# revision 21
# baseline (speedup 1.0000x reference)
"""Trainium2 Bass kernel for nn_BackwardConv2D (batched transposed conv vjp).

Math (per batch b, per tensor t in {u,l}):
  W = w_out_t[b,0]  viewed as [65536, 32]  (row = s*64+co, s=y*32+x spatial, col = j)
  G[t_out*64+ci, j] = sum_{dy,dx,co} Kf[dy,dx,co,ci] * W[s_in*64+co, j]
     where s_in = (y_o+dy-1)*32 + (x_o+dx-1), Kf[dy,dx,co,ci] = kernel[2-dy,2-dx,ci,co]
  bias_out[b,0,j] = b_out_t[b,0,j] + sum_{s,c} bias[c] * W[s*64+c, j]

Sharding: data-parallel over B: core c handles (w_out_u[c], w_out_l[c]). The
tiny bias reduction (0.001% of FLOPs) runs on the host at full fp precision;
the device runs the conv only.

Host pre/post: W is cast to bf16 and pre-permuted to the SBUF block layout
  wp[p, m, j] = W[m*128+p, j]   (contiguous [128, 512, 32] -> max-rate DMA;
a strided on-device gather is descriptor-rate-bound at ~90 GB/s). The device
writes G in the same block layout; the host un-permutes.

Device: per output row y_o and tap row dy, ONE fused K=128/M=128/N=512 bf16
matmul covers the 4 taps whose input x' falls in the same 128-row block window
(out partitions 0-63 = even x_o ci, 64-127 = odd x_o ci). The 2 leftover taps
are K=64/M=64 singles in opposite (row,col) array quadrants; their LDWEIGHTS
land in the quadrant the other single is not streaming through, so each
(e,o) single pair dual-issues — measured ~92% of this scheme's PE ideal.
x-boundary: only the singles can go invalid; shrink their N range. y-boundary:
skip dy out of range. PSUM accumulates all taps; one [128,512] copy per y_o
(alternating DVE/ACT) into 2-row staging tiles, contiguous DMA out.
"""

import sys

sys.path.insert(0, "/opt/trn_rl_repo")

import numpy as np

B, H, W_, CIN = 8, 32, 32, 64
COUT, NOUT = 64, 32
NFLAT = H * W_ * CIN  # 65536
NCORES = 8
NM = NFLAT // 128  # 512 blocks of 128 rows

_cache = {}


def _build():
    from concourse import bacc, bass, tile, mybir

    f32 = mybir.dt.float32
    bf16 = mybir.dt.bfloat16
    nc = bacc.Bacc("TRN2", target_bir_lowering=False, debug=False)

    w_dram = [
        nc.dram_tensor("w0", (128, NM, 32), bf16, kind="ExternalInput"),
        nc.dram_tensor("w1", (128, NM, 32), bf16, kind="ExternalInput"),
    ]
    lhs_pairs = nc.dram_tensor("lhs_pairs", (128, 3, 128), bf16, kind="ExternalInput")
    lhs_singles = nc.dram_tensor("lhs_singles", (128, 3, 64), bf16, kind="ExternalInput")
    g_dram = [
        nc.dram_tensor("g0", (128, NM, 32), f32, kind="ExternalOutput"),
        nc.dram_tensor("g1", (128, NM, 32), f32, kind="ExternalOutput"),
    ]

    with tile.TileContext(nc) as tc:
        with (
            tc.tile_pool(name="constp", bufs=1) as constp,
            tc.tile_pool(name="slabp", bufs=2) as slabp,
            tc.tile_pool(name="stagp", bufs=8) as stagp,
            tc.tile_pool(name="psump", bufs=8, space="PSUM") as psump,
        ):
            lp = constp.tile([128, 3, 128], bf16, name="lp")
            nc.scalar.dma_start(lp[:, :, :], lhs_pairs[:, :, :])
            ls = constp.tile([128, 3, 64], bf16, name="ls")
            nc.scalar.dma_start(ls[:, :, :], lhs_singles[:, :, :])

            # HAM warm-up: dummy matmuls with no DMA dependency run during the
            # first slab chunk's load, so real matmuls start at the full 2.4 GHz
            warm = constp.tile([128, 640], bf16, name="warm")
            nc.gpsimd.memset(warm[:, :], 0.0)
            wps = psump.tile([128, 16, 32], f32, name="ps", tag="ps")
            for _ in range(18):
                nc.tensor.matmul(
                    wps[:, :, :],
                    warm[:, 0:128],
                    warm[:, 128:640],
                    start=True,
                    stop=True,
                    skip_group_check=True,
                )

            for si in range(2):
                slab = slabp.tile([128, NM, 32], bf16, name="slab", tag="slab")
                # contiguous in-DMA, chunked so the first y-blocks start early
                for lo, hi in ((0, 48), (48, 112), (112, 272), (272, NM)):
                    nc.sync.dma_start(slab[:, lo:hi, :], w_dram[si][:, lo:hi, :])

                blocks = [[0, 1]] + [list(range(lo, lo + 6)) for lo in range(2, H, 6)]
                for blk in blocks:
                    ptiles = {
                        y: psump.tile([128, 16, 32], f32, name="ps", tag="ps")
                        for y in blk
                    }
                    valid = {y: [dy for dy in range(3) if 0 <= y + dy - 1 < H] for y in blk}
                    # fused pairs: K=128, M=128, N=512 (dy-major: lhsT reuse)
                    for dy in range(3):
                        for y in blk:
                            if dy not in valid[y]:
                                continue
                            yp = y + dy - 1
                            nc.tensor.matmul(
                                ptiles[y][:, :, :],
                                lp[:, dy, :],
                                slab[:, yp * 16 : (yp + 1) * 16, :],
                                start=dy == valid[y][0],
                                stop=False,
                                skip_group_check=True,
                            )
                    # singles (K=64, M=64, N=480, opposite quadrants) y-pair-major
                    # with the copy right after, so PSUM slots recycle steadily
                    for i in range(0, len(blk), 2):
                        pair = blk[i : i + 2]
                        for y in pair:
                            for dy in valid[y]:
                                yp = y + dy - 1
                                last = dy == valid[y][-1]
                                # even x_o (>=2), tap dx=0 reads odd s -> rows 64-127
                                nc.tensor.matmul(
                                    ptiles[y][0:64, 1:16, :],
                                    ls[64:128, dy, :],
                                    slab[64:128, yp * 16 : yp * 16 + 15, :],
                                    start=False,
                                    stop=last,
                                    skip_group_check=True,
                                )
                                # odd x_o (<=29), tap dx=2 reads even s -> rows 0-63
                                nc.tensor.matmul(
                                    ptiles[y][64:128, 0:15, :],
                                    ls[0:64, dy, :],
                                    slab[0:64, yp * 16 + 1 : (yp + 1) * 16, :],
                                    start=False,
                                    stop=last,
                                    skip_group_check=True,
                                )
                        y0, y1 = pair
                        stag = stagp.tile([128, 32, 32], f32, name="stag", tag="stag")
                        nc.vector.tensor_copy(stag[:, 0:16, :], ptiles[y0][:, :, :])
                        nc.scalar.copy(stag[:, 16:32, :], ptiles[y1][:, :, :])
                        dma_eng = nc.sync if (y0 // 2) % 2 == 0 else nc.scalar
                        dma_eng.dma_start(
                            g_dram[si][:, y0 * 16 : (y1 + 1) * 16, :], stag[:, :, :]
                        )

    nc.compile()
    return nc


def _host_prep(kernel_np):
    import ml_dtypes

    bf = ml_dtypes.bfloat16
    # Kf[dy,dx,co,ci] = kernel[2-dy,2-dx,ci,co]
    kf = np.transpose(kernel_np[::-1, ::-1, :, :], (0, 1, 3, 2)).astype(np.float32)
    lhs_pairs = np.zeros((128, 3, 128), np.float32)
    lhs_singles = np.zeros((128, 3, 64), np.float32)
    for dy in range(3):
        # fused pair: rows = input (x'=2e on 0-63, x'=2e+1 on 64-127),
        # cols = output ci (even x_o on 0-63, odd x_o on 64-127)
        lhs_pairs[0:64, dy, 0:64] = kf[dy, 1]
        lhs_pairs[64:128, dy, 0:64] = kf[dy, 2]
        lhs_pairs[0:64, dy, 64:128] = kf[dy, 0]
        lhs_pairs[64:128, dy, 64:128] = kf[dy, 1]
        # singles: parts 0-63 used by odd x_o tap dx=2; 64-127 by even x_o tap dx=0
        lhs_singles[0:64, dy] = kf[dy, 2]
        lhs_singles[64:128, dy] = kf[dy, 0]
    return lhs_pairs.astype(bf), lhs_singles.astype(bf)


def _to_block_layout(w, bf):
    # [65536, 32] f32 -> [128, 512, 32] bf16 with wp[p,m,j] = W[m*128+p, j]
    return np.ascontiguousarray(
        w.astype(bf).reshape(NM, 128, NOUT).transpose(1, 0, 2)
    )


def _from_block_layout(g):
    # [128, 512, 32] f32 -> [65536, 32]
    return g.transpose(1, 0, 2).reshape(NFLAT, NOUT)


def _host_bias(w, b_out, bias):
    # b[b,0,j] = b_out[b,0,j] + sum_{s,c} bias[c] * w[b,0,s*64+c,j]
    s = w[:, 0].reshape(B, H * W_, CIN, NOUT).sum(axis=1)  # [B, c, j]
    return (b_out[:, 0] + np.einsum("c,bcj->bj", bias, s))[:, None, :].astype(np.float32)


def _run(inputs, trace=False):
    import ml_dtypes

    from concourse import bass_utils

    bf = ml_dtypes.bfloat16
    if "nc" not in _cache:
        _cache["nc"] = _build()
    nc = _cache["nc"]

    lhs_pairs, lhs_singles = _host_prep(np.asarray(inputs["kernel"], np.float32))
    wu = np.asarray(inputs["w_out_u"], np.float32)
    wl = np.asarray(inputs["w_out_l"], np.float32)
    bias = np.asarray(inputs["bias"], np.float32)

    in_maps = []
    for c in range(NCORES):
        in_maps.append(
            {
                "w0": _to_block_layout(wu[c, 0], bf),
                "w1": _to_block_layout(wl[c, 0], bf),
                "lhs_pairs": lhs_pairs,
                "lhs_singles": lhs_singles,
            }
        )

    b_u = _host_bias(wu, np.asarray(inputs["b_out_u"], np.float32), bias)
    b_l = _host_bias(wl, np.asarray(inputs["b_out_l"], np.float32), bias)

    res = bass_utils.run_bass_kernel_spmd(
        nc, in_maps, core_ids=list(range(NCORES)), trace=trace
    )

    w_u_out = np.empty((B, 1, NFLAT, NOUT), np.float32)
    w_l_out = np.empty((B, 1, NFLAT, NOUT), np.float32)
    for c in range(NCORES):
        w_u_out[c, 0] = _from_block_layout(res.results[c]["g0"])
        w_l_out[c, 0] = _from_block_layout(res.results[c]["g1"])
    return (w_u_out, b_u, w_l_out, b_l), res


def kernel(**inputs):
    outs, _ = _run(inputs, trace=False)
    return outs


# revision 22
# speedup vs baseline: 1.0185x; 1.0185x over previous
"""Trainium2 Bass kernel for nn_BackwardConv2D (batched transposed conv vjp).

Math (per batch b, per tensor t in {u,l}):
  W = w_out_t[b,0]  viewed as [65536, 32]  (row = s*64+co, s=y*32+x spatial, col = j)
  G[t_out*64+ci, j] = sum_{dy,dx,co} Kf[dy,dx,co,ci] * W[s_in*64+co, j]
     where s_in = (y_o+dy-1)*32 + (x_o+dx-1), Kf[dy,dx,co,ci] = kernel[2-dy,2-dx,ci,co]
  bias_out[b,0,j] = b_out_t[b,0,j] + sum_{s,c} bias[c] * W[s*64+c, j]

Sharding: data-parallel over B: core c handles (w_out_u[c], w_out_l[c]). The
tiny bias reduction (0.001% of FLOPs) runs on the host at full fp precision;
the device runs the conv only.

Host pre/post: W is cast to bf16 and pre-permuted to the SBUF block layout
  wp[p, m, j] = W[m*128+p, j]   (contiguous [128, 512, 32] -> max-rate DMA;
a strided on-device gather is descriptor-rate-bound at ~90 GB/s). The device
writes G in the same block layout; the host un-permutes.

Device: per output row y_o and tap row dy, ONE fused K=128/M=128/N=512 bf16
matmul covers the 4 taps whose input x' falls in the same 128-row block window
(out partitions 0-63 = even x_o ci, 64-127 = odd x_o ci). The 2 leftover taps
are K=64/M=64 singles in opposite (row,col) array quadrants; their LDWEIGHTS
land in the quadrant the other single is not streaming through, so each
(e,o) single pair dual-issues — measured ~92% of this scheme's PE ideal.
x-boundary: only the singles can go invalid; shrink their N range. y-boundary:
skip dy out of range. PSUM accumulates all taps; one [128,512] copy per y_o
(alternating DVE/ACT) into 2-row staging tiles, contiguous DMA out.
"""

import sys

sys.path.insert(0, "/opt/trn_rl_repo")

import numpy as np

B, H, W_, CIN = 8, 32, 32, 64
COUT, NOUT = 64, 32
NFLAT = H * W_ * CIN  # 65536
NCORES = 8
NM = NFLAT // 128  # 512 blocks of 128 rows

_cache = {}


def _build():
    from concourse import bacc, bass, tile, mybir

    f32 = mybir.dt.float32
    bf16 = mybir.dt.bfloat16
    nc = bacc.Bacc("TRN2", target_bir_lowering=False, debug=False)

    w_dram = [
        nc.dram_tensor("w0", (128, NM, 32), bf16, kind="ExternalInput"),
        nc.dram_tensor("w1", (128, NM, 32), bf16, kind="ExternalInput"),
    ]
    lhs_pairs = nc.dram_tensor("lhs_pairs", (128, 3, 128), bf16, kind="ExternalInput")
    lhs_singles = nc.dram_tensor("lhs_singles", (128, 3, 64), bf16, kind="ExternalInput")
    g_dram = [
        nc.dram_tensor("g0", (128, NM, 32), f32, kind="ExternalOutput"),
        nc.dram_tensor("g1", (128, NM, 32), f32, kind="ExternalOutput"),
    ]

    with tile.TileContext(nc) as tc:
        with (
            tc.tile_pool(name="constp", bufs=1) as constp,
            tc.tile_pool(name="slabp", bufs=2) as slabp,
            tc.tile_pool(name="stagp", bufs=8) as stagp,
            tc.tile_pool(name="psump", bufs=8, space="PSUM") as psump,
        ):
            lp = constp.tile([128, 3, 128], bf16, name="lp")
            nc.scalar.dma_start(lp[:, :, :], lhs_pairs[:, :, :])
            ls = constp.tile([128, 3, 64], bf16, name="ls")
            nc.scalar.dma_start(ls[:, :, :], lhs_singles[:, :, :])

            # HAM warm-up: dummy matmuls with no DMA dependency run during the
            # first slab chunk's load, so real matmuls start at the full 2.4 GHz
            warm = constp.tile([128, 640], bf16, name="warm")
            nc.gpsimd.memset(warm[:, :], 0.0)
            wps = psump.tile([128, 16, 32], f32, name="ps", tag="ps")
            for _ in range(18):
                nc.tensor.matmul(
                    wps[:, :, :],
                    warm[:, 0:128],
                    warm[:, 128:640],
                    start=True,
                    stop=True,
                    skip_group_check=True,
                )

            for si in range(2):
                slab = slabp.tile([128, NM, 32], bf16, name="slab", tag="slab")
                # contiguous in-DMA, chunked so the first y-blocks start early
                for lo, hi in ((0, 48), (48, 112), (112, 272), (272, NM)):
                    nc.sync.dma_start(slab[:, lo:hi, :], w_dram[si][:, lo:hi, :])

                blocks = [[0, 1]] + [list(range(lo, lo + 6)) for lo in range(2, H, 6)]
                for blk in blocks:
                    ptiles = {
                        y: psump.tile([128, 16, 32], f32, name="ps", tag="ps")
                        for y in blk
                    }
                    valid = {y: [dy for dy in range(3) if 0 <= y + dy - 1 < H] for y in blk}
                    # fused pairs: K=128, M=128, N=512 (dy-major: lhsT reuse)
                    for dy in range(3):
                        for y in blk:
                            if dy not in valid[y]:
                                continue
                            yp = y + dy - 1
                            nc.tensor.matmul(
                                ptiles[y][:, :, :],
                                lp[:, dy, :],
                                slab[:, yp * 16 : (yp + 1) * 16, :],
                                start=dy == valid[y][0],
                                stop=False,
                                skip_group_check=True,
                            )
                    # singles (K=64, M=64, N=480, opposite quadrants) y-pair-major
                    # with the copy right after, so PSUM slots recycle steadily
                    for i in range(0, len(blk), 2):
                        pair = blk[i : i + 2]
                        for y in pair:
                            for dy in valid[y]:
                                yp = y + dy - 1
                                last = dy == valid[y][-1]
                                # even x_o (>=2), tap dx=0 reads odd s -> rows 64-127
                                nc.tensor.matmul(
                                    ptiles[y][0:64, 1:16, :],
                                    ls[64:128, dy, :],
                                    slab[64:128, yp * 16 : yp * 16 + 15, :],
                                    start=False,
                                    stop=last,
                                    skip_group_check=True,
                                )
                                # odd x_o (<=29), tap dx=2 reads even s -> rows 0-63
                                nc.tensor.matmul(
                                    ptiles[y][64:128, 0:15, :],
                                    ls[0:64, dy, :],
                                    slab[0:64, yp * 16 + 1 : (yp + 1) * 16, :],
                                    start=False,
                                    stop=last,
                                    skip_group_check=True,
                                )
                        y0, y1 = pair
                        stag = stagp.tile([128, 32, 32], f32, name="stag", tag="stag")
                        nc.vector.tensor_copy(stag[:, 0:16, :], ptiles[y0][:, :, :])
                        nc.scalar.copy(stag[:, 16:32, :], ptiles[y1][:, :, :])
                        nc.scalar.dma_start(
                            g_dram[si][:, y0 * 16 : (y1 + 1) * 16, :], stag[:, :, :]
                        )

    nc.compile()
    return nc


def _host_prep(kernel_np):
    import ml_dtypes

    bf = ml_dtypes.bfloat16
    # Kf[dy,dx,co,ci] = kernel[2-dy,2-dx,ci,co]
    kf = np.transpose(kernel_np[::-1, ::-1, :, :], (0, 1, 3, 2)).astype(np.float32)
    lhs_pairs = np.zeros((128, 3, 128), np.float32)
    lhs_singles = np.zeros((128, 3, 64), np.float32)
    for dy in range(3):
        # fused pair: rows = input (x'=2e on 0-63, x'=2e+1 on 64-127),
        # cols = output ci (even x_o on 0-63, odd x_o on 64-127)
        lhs_pairs[0:64, dy, 0:64] = kf[dy, 1]
        lhs_pairs[64:128, dy, 0:64] = kf[dy, 2]
        lhs_pairs[0:64, dy, 64:128] = kf[dy, 0]
        lhs_pairs[64:128, dy, 64:128] = kf[dy, 1]
        # singles: parts 0-63 used by odd x_o tap dx=2; 64-127 by even x_o tap dx=0
        lhs_singles[0:64, dy] = kf[dy, 2]
        lhs_singles[64:128, dy] = kf[dy, 0]
    return lhs_pairs.astype(bf), lhs_singles.astype(bf)


def _to_block_layout(w, bf):
    # [65536, 32] f32 -> [128, 512, 32] bf16 with wp[p,m,j] = W[m*128+p, j]
    return np.ascontiguousarray(
        w.astype(bf).reshape(NM, 128, NOUT).transpose(1, 0, 2)
    )


def _from_block_layout(g):
    # [128, 512, 32] f32 -> [65536, 32]
    return g.transpose(1, 0, 2).reshape(NFLAT, NOUT)


def _host_bias(w, b_out, bias):
    # b[b,0,j] = b_out[b,0,j] + sum_{s,c} bias[c] * w[b,0,s*64+c,j]
    s = w[:, 0].reshape(B, H * W_, CIN, NOUT).sum(axis=1)  # [B, c, j]
    return (b_out[:, 0] + np.einsum("c,bcj->bj", bias, s))[:, None, :].astype(np.float32)


def _run(inputs, trace=False):
    import ml_dtypes

    from concourse import bass_utils

    bf = ml_dtypes.bfloat16
    if "nc" not in _cache:
        _cache["nc"] = _build()
    nc = _cache["nc"]

    lhs_pairs, lhs_singles = _host_prep(np.asarray(inputs["kernel"], np.float32))
    wu = np.asarray(inputs["w_out_u"], np.float32)
    wl = np.asarray(inputs["w_out_l"], np.float32)
    bias = np.asarray(inputs["bias"], np.float32)

    in_maps = []
    for c in range(NCORES):
        in_maps.append(
            {
                "w0": _to_block_layout(wu[c, 0], bf),
                "w1": _to_block_layout(wl[c, 0], bf),
                "lhs_pairs": lhs_pairs,
                "lhs_singles": lhs_singles,
            }
        )

    b_u = _host_bias(wu, np.asarray(inputs["b_out_u"], np.float32), bias)
    b_l = _host_bias(wl, np.asarray(inputs["b_out_l"], np.float32), bias)

    res = bass_utils.run_bass_kernel_spmd(
        nc, in_maps, core_ids=list(range(NCORES)), trace=trace
    )

    w_u_out = np.empty((B, 1, NFLAT, NOUT), np.float32)
    w_l_out = np.empty((B, 1, NFLAT, NOUT), np.float32)
    for c in range(NCORES):
        w_u_out[c, 0] = _from_block_layout(res.results[c]["g0"])
        w_l_out[c, 0] = _from_block_layout(res.results[c]["g1"])
    return (w_u_out, b_u, w_l_out, b_l), res


def kernel(**inputs):
    outs, _ = _run(inputs, trace=False)
    return outs
